# revision 88
# baseline (speedup 1.0000x reference)
"""AttnBlock (GroupNorm + single-head attention + proj + residual) on 8 trn2 cores.

Sharding: core = (image b, query-half h).  Each core gets the full 4096-pixel
image (queries permuted to rows 0:2048), computes GroupNorm + K/V for the whole
image, attention/proj for its 2048 queries, and writes a (2048, 512) shard.

Math folding done on host (all exact, fp32):
  h = gn(x)*gamma + beta ; q = h@wq+bq ; ... ; out = x + attn@wp + bp
  => with xn = (x-mu_g)*rstd_g (pure normalize):
     q' = (xn@wq_f + bq_f)          wq_f = diag(gamma)@wq/sqrt(C), bq_f = (beta@wq+bq)/sqrt(C)
     k  = (xn@wk_f + bk_f)          wk_f = diag(gamma)@wk,        bk_f = beta@wk+bk
     v' = xn@wv_f (bias dropped)    wv_f = diag(gamma)@wv
     out = x + softmax(q'k^T)@v'@wp + bfin,   bfin = (beta@wv+bv)@wp + bp
  (softmax rows sum to 1, so the v-bias passes through attention unchanged.)

Engine plan (cost-model driven):
  - Scores are computed TRANSPOSED (S^T[k,q] tiles) so exp() on the Act engine
    writes P^T to SBUF fp8 directly -- no PE transposes / DVE copies of P.
  - Softmax denominators Z come from N=1 matmuls (P^T-block stationary, ones
    rhs) interleaved with the PV matmuls; N=1 matmuls are ~free on PE.
  - 1/Z is applied on the PV psum during the fp8 cast of the attention output.
  - Epilogues are split across Pool (gpsimd, 0.83ns/el, no psum penalty) and
    DVE; all DMA issues from the otherwise idle SP (sync) engine.
"""

import sys

sys.path.insert(0, "/opt/trn_rl_repo")

import numpy as np
import ml_dtypes

import concourse.bass as bass
import concourse.tile as tile
from concourse import mybir
from concourse.masks import make_identity

F32 = mybir.dt.float32
BF16 = mybir.dt.bfloat16
FP8 = mybir.dt.float8e4
W8SCALE = 64.0
AF = mybir.ActivationFunctionType
ALU = mybir.AluOpType
AX = mybir.AxisListType
DR = mybir.MatmulPerfMode.DoubleRow

PIX = 4096          # 64*64 pixels per image
QPIX = 2048         # queries per core
C = 512             # channels
NCH = 4             # channel chunks of 128
NQT = QPIX // 128   # 16 query tiles
NKB = PIX // 128    # 32 key blocks
NSL = PIX // 512    # 8 pixel slices of 512
QCH = 1024          # attention query chunk (exp tile width)
NCHUNK = QPIX // QCH
EPS = 1e-5
EBIAS = -2.772589   # ln(1/16): keeps exp() in fp8-e4m3 range; cancels in softmax
DS = 1.0 / W8SCALE

_CACHED = {}


def build_program(spill=True):
    nc = bass.Bass()

    x = nc.dram_tensor("x", [QPIX, C], F32, kind="ExternalInput").ap()
    xt_in = nc.dram_tensor("xt", [C, PIX], FP8, kind="ExternalInput").ap()
    wq = nc.dram_tensor("wq", [C, C], FP8, kind="ExternalInput").ap()
    wk = nc.dram_tensor("wk", [C, C], FP8, kind="ExternalInput").ap()
    wv = nc.dram_tensor("wv", [C, C], FP8, kind="ExternalInput").ap()
    wp = nc.dram_tensor("wp", [C, C], FP8, kind="ExternalInput").ap()
    bq = nc.dram_tensor("bq", [C], F32, kind="ExternalInput").ap()
    bk = nc.dram_tensor("bk", [C], F32, kind="ExternalInput").ap()
    bfin = nc.dram_tensor("bfin", [C], F32, kind="ExternalInput").ap()
    gmask = nc.dram_tensor("gmask", [128, 128], F32, kind="ExternalInput").ap()
    out = nc.dram_tensor("out", [QPIX, C], F32, kind="ExternalOutput").ap()

    with tile.TileContext(nc) as tc:
        with (
            tc.tile_pool(name="singles", bufs=1) as singles,
            tc.tile_pool(name="big", bufs=1) as big,
            tc.tile_pool(name="stats", bufs=4) as stats,
            tc.tile_pool(name="work", bufs=3) as work,
            tc.tile_pool(name="ps", bufs=1, space="PSUM") as ps,
        ):
            # ---- constants / weights (all DMA via the idle SP engine) ----
            ident_b = singles.tile([128, 128], BF16, tag="idb")
            make_identity(nc, ident_b)
            gmask_sb = singles.tile([128, 128], F32, tag="gmask")
            nc.sync.dma_start(out=gmask_sb, in_=gmask)
            eps_sb = singles.tile([128, 1], F32, tag="eps")
            nc.vector.memset(eps_sb, EPS)
            ebias_sb = singles.tile([128, 1], F32, tag="ebias")
            nc.vector.memset(ebias_sb, EBIAS)
            ones2 = singles.tile([128, 2, 1], FP8, tag="ones2")
            nc.vector.memset(ones2, 1.0)

            # ---- phase 0: input DMAs (xt first: stats need it asap).
            # ci order (1,0,2,3): ci0's moments run on Act, so DVE's first
            # bn_stats chunk (ci1) ships first to start both engines early.
            xh = big.tile([128, NCH, PIX], FP8, tag="xh")     # 16KB/part
            for ci in (1, 0, 2, 3):
                for s4 in range(4):
                    nc.sync.dma_start(
                        out=xh[:, ci, s4 * 1024:(s4 + 1) * 1024],
                        in_=xt_in[ci * 128:(ci + 1) * 128,
                                  s4 * 1024:(s4 + 1) * 1024],
                    )
            w_sb = {}
            for name, ap in (("wk", wk), ("wq", wq), ("wv", wv), ("wp", wp)):
                t = singles.tile([128, NCH, C], FP8, tag=name, name=name)
                nc.sync.dma_start(
                    out=t, in_=ap.rearrange("(ci p) co -> p ci co", p=128)
                )
                w_sb[name] = t
            bq_sb = singles.tile([128, NCH], F32, tag="bq")
            nc.sync.dma_start(out=bq_sb, in_=bq.rearrange("(c p) -> p c", p=128))
            bk_sb = singles.tile([128, NCH], F32, tag="bk")
            nc.sync.dma_start(out=bk_sb, in_=bk.rearrange("(c p) -> p c", p=128))
            bfin_bc = singles.tile([128, C], F32, tag="bfin")
            nc.sync.dma_start(
                out=bfin_bc,
                in_=bass.AP(tensor=bfin.tensor, offset=bfin.offset,
                            ap=[[0, 128], [1, C]]),
            )
            # residual rows for this core's queries
            xq_all = big.tile([128, NQT, C], F32, tag="xq")   # 32KB/part
            nc.sync.dma_start(
                out=xq_all,
                in_=x.rearrange("(t p) c -> p t c", p=128),
            )

            # ---- persistent big tensors ----
            h = big.tile([128, NCH, PIX], FP8, tag="h")       # 16KB/part
            kT = big.tile([128, NCH, PIX], FP8, tag="kT")     # 16KB/part
            qT = big.tile([128, NCH, QPIX], FP8, tag="qT")    # 8KB/part
            V = big.tile([128, NKB, C], FP8, tag="V")         # 16KB/part
            PT = big.tile([128, NKB, QPIX], FP8, tag="PT")    # 64KB/part
            rcp_all = stats.tile([128, NQT], F32, tag="rcp", bufs=1)
            bn6 = stats.tile([128, NCH, NSL, 6], F32, tag="bn6", bufs=1)

            # ---- phase 1: GroupNorm stats (per channel chunk) ----
            # ci 1-3: DVE bn_stats.  ci 0 (first to arrive): raw moments on
            # the otherwise-idle Act head (Square/Identity with accum_out) so
            # the DVE serial stats chain shortens by a quarter.
            # DVE does only bn_stats/aggr (its serial chain is the head
            # gate).  The c_in chunk order is PERMUTED to (1,2,3,0) in h and
            # in the kq/v weight rows (host side), so the (plane0,plane1) DR
            # contraction pair = (ci1,ci2) whose stats finish first: half of
            # every kq gemm pre-runs before ci3/ci0 chains land.  Chain ops
            # run in two groups: A=(ci1,ci2), B=(ci3,ci0).
            POS = {1: 0, 2: 1, 3: 2, 0: 3}
            me_all = stats.tile([128, NCH, 2], F32, tag="me", bufs=1)
            gst_all = stats.tile([128, NCH, 2], F32, tag="gst", bufs=1)
            vgs = stats.tile([128, NCH], F32, tag="vg", bufs=1)
            lnv = stats.tile([128, NCH], F32, tag="lnv", bufs=1)
            rstd_all = stats.tile([128, NCH], F32, tag="rstd", bufs=1)

            def chain_group(g, eng):  # g=0: pos 0,1  /  g=1: pos 2,3
                s2 = slice(2 * g, 2 * g + 2)
                gps = ps.tile([128, 512], F32, tag="pv", name=f"gps{g}")
                nc.tensor.matmul(
                    gps[:, 0:4], gmask_sb,
                    me_all[:, s2, :].rearrange("p a b -> p (a b)"),
                    start=True, stop=True)
                nc.vector.tensor_copy(  # gpsimd may not touch psum
                    gst_all[:, s2, :],
                    gps[:, 0:4].rearrange("p (a b) -> p a b", a=2))
                eng.tensor_mul(vgs[:, s2], gst_all[:, s2, 0],
                               gst_all[:, s2, 0])
                eng.tensor_sub(vgs[:, s2], vgs[:, s2],
                               gst_all[:, s2, 1])
                # rstd = (var+eps)^-0.5 via Ln+Exp (vgs holds -var): same
                # activation table as Identity/Square/Exp -> no reloads.
                nc.scalar.activation(out=lnv[:, s2], in_=vgs[:, s2],
                                     func=AF.Ln, bias=eps_sb, scale=-1.0)
                nc.scalar.activation(out=rstd_all[:, s2], in_=lnv[:, s2],
                                     func=AF.Exp, scale=-0.5)

            for ci in (1, 2, 3, 0):
                pos = POS[ci]
                if ci != 0:
                    for sl in range(NSL):
                        nc.vector.bn_stats(
                            out=bn6[:, ci, sl, :],
                            in_=xh[:, ci, sl * 512:(sl + 1) * 512],
                        )
                    mv = stats.tile([128, 2], F32, tag="mv")
                    nc.vector.bn_aggr(out=mv, in_=bn6[:, ci, :, :])
                    meng = nc.vector if ci == 3 else nc.gpsimd
                    meng.tensor_copy(me_all[:, pos, 0:1], mv[:, 0:1])
                    meng.tensor_mul(me_all[:, pos, 1:2], mv[:, 0:1],
                                    mv[:, 0:1])
                    meng.tensor_add(me_all[:, pos, 1:2],
                                    me_all[:, pos, 1:2], mv[:, 1:2])
                else:
                    # half-row moments on Act, pipelined behind the ci0 DMA
                    # chunks; the dummy outs land in the h region that
                    # normalize overwrites right after.
                    sx = stats.tile([128, 2, 2], F32, tag="sx", bufs=1)
                    for hf in range(2):
                        sl = slice(hf * 2048, (hf + 1) * 2048)
                        nc.scalar.activation(
                            out=h[:, pos, sl], in_=xh[:, 0, sl],
                            func=AF.Identity, accum_out=sx[:, 0, hf:hf + 1])
                        nc.scalar.activation(
                            out=h[:, pos, sl], in_=xh[:, 0, sl],
                            func=AF.Square, accum_out=sx[:, 1, hf:hf + 1])
                    nc.gpsimd.tensor_add(me_all[:, pos, :], sx[:, :, 0],
                                         sx[:, :, 1])
                    nc.gpsimd.tensor_scalar_mul(out=me_all[:, pos, :],
                                                in0=me_all[:, pos, :],
                                                scalar1=1.0 / PIX)
                if ci == 2:
                    chain_group(0, nc.gpsimd)
                elif ci == 0:
                    chain_group(1, nc.vector)

            def norm_block(ci, b4, eng):
                """normalize + fp8 cast of one 1024-pix block of chunk ci."""
                pos = POS[ci]
                eng.tensor_scalar(
                    out=h[:, pos, b4 * 1024:(b4 + 1) * 1024],
                    in0=xh[:, ci, b4 * 1024:(b4 + 1) * 1024],
                    scalar1=gst_all[:, pos, 0:1],
                    scalar2=rstd_all[:, pos:pos + 1],
                    op0=ALU.subtract, op1=ALU.mult,
                )

            for ci in (1, 2, 3, 0):    # queries block first (all Pool: it
                norm_block(ci, 0, nc.gpsimd)  # only does sbuf<->sbuf work)

            # ---- gemm helpers (all fp8 DoubleRow) ----
            # kq gemms rotate psum tags pv/pj (Pool-drained); V uses at
            # (DVE/Pool-drained) so the two epilogue chains never couple.
            gidx = [0]
            HROT = ("pv", "pj", "at", "z", "sT", "sT")  # sT banks idle pre-S^T

            def gpsum(cols, tag=None, head=False):
                if tag is None:
                    tag = HROT[gidx[0] % 6] if head else \
                        ("pv", "pj", "at", "z")[gidx[0] % 4]
                gidx[0] += 1
                kw = {"bufs": 2} if tag == "sT" else {}
                t = ps.tile([128, cols], F32, tag=tag,
                            name=f"g{gidx[0]}_{tag}", **kw)
                return t

            def kq_gemm(wname, dst, bias_sb, sl, co, epi_eng, head=False):
                """dst[:, co, sl*512:...] = (w^T @ h)*DS + bias  (out [c_out, pix])"""
                p = gpsum(512, head=head)
                for u in range(2):
                    nc.tensor.matmul(
                        p,
                        w_sb[wname][:, 2 * u:2 * u + 2, co * 128:(co + 1) * 128],
                        h[:, 2 * u:2 * u + 2, sl * 512:(sl + 1) * 512],
                        start=(u == 0), stop=(u == 1), perf_mode=DR,
                    )
                if epi_eng is nc.scalar:  # head epis on the idle Act
                    nc.scalar.activation(
                        out=dst[:, co, sl * 512:(sl + 1) * 512], in_=p,
                        func=AF.Identity, scale=DS,
                        bias=bias_sb[:, co:co + 1])
                else:
                    epi_eng.tensor_scalar(
                        out=dst[:, co, sl * 512:(sl + 1) * 512],
                        in0=p, scalar1=DS, scalar2=bias_sb[:, co:co + 1],
                        op0=ALU.mult, op1=ALU.add,
                    )

            def v_gemm(kb, epi_eng):
                """V[:, kb, :] = (h-block @ wv)*DS  (out [pix, c])"""
                p = gpsum(512, tag=("at", "pj")[kb % 2])
                for u in range(2):
                    nc.tensor.matmul(
                        p,
                        h[:, 2 * u:2 * u + 2, kb * 128:(kb + 1) * 128],
                        w_sb["wv"][:, 2 * u:2 * u + 2, :],
                        start=(u == 0), stop=(u == 1), perf_mode=DR,
                    )
                epi_eng.tensor_scalar_mul(out=V[:, kb, :], in0=p, scalar1=DS)

            # ---- phase 2 (head): exactly what S^T(kb0) needs, first, with a
            # 6-deep psum rotation (sT banks are idle until the S^T sweep).
            # Head epis split Act (idle pre-exp) / DVE.
            for co in range(NCH):                 # kT sl0 gates S^T(kb0):
                kq_gemm("wk", kT, bk_sb, 0, co,   # keep its epis off the
                        nc.vector, head=True)     # Act queue
            for sl in range(QCH // 512):          # qT for chunk 0
                for co in range(NCH):
                    kq_gemm("wq", qT, bq_sb, sl, co,
                            nc.scalar if co < 2 else nc.vector, head=True)
            for co in range(NCH):
                kq_gemm("wk", kT, bk_sb, 1, co, nc.vector)

            # ---- attention ----
            def st_exp(chunk, kb):
                """S^T tile [128k, QCH] for key-block kb, then exp -> PT fp8."""
                qb = chunk * QCH
                sps = ps.tile([128, QCH], F32, tag="sT", bufs=2,
                              name=f"sps{chunk}_{kb}")
                for qh in range(QCH // 512):
                    for u in range(2):
                        nc.tensor.matmul(
                            sps[:, qh * 512:(qh + 1) * 512],
                            kT[:, 2 * u:2 * u + 2, kb * 128:(kb + 1) * 128],
                            qT[:, 2 * u:2 * u + 2,
                               qb + qh * 512:qb + (qh + 1) * 512],
                            start=(u == 0), stop=(u == 1), perf_mode=DR,
                        )
                nc.scalar.activation(
                    out=PT[:, kb, qb:qb + QCH], in_=sps,
                    func=AF.Exp, scale=1.0, bias=ebias_sb,
                )

            zcols = ps.tile([128, NQT], F32, tag="z", name="zcols")

            def pv_part(qt, pv_tag="pv"):
                """V-stationary PV: out^T[c, q] lands pre-transposed in one
                column-sliced psum bank; Z rides as free N=1 matmuls."""
                pv = ps.tile([128, C], F32, tag=pv_tag,
                             bufs=2 if pv_tag == "sT" else None, name=f"pv{qt}")
                qs = slice(qt * 128, (qt + 1) * 128)
                for cj in range(NCH):
                    for j in range(NKB // 2):
                        nc.tensor.matmul(
                            pv[:, cj * 128:(cj + 1) * 128],
                            V[:, 2 * j:2 * j + 2, cj * 128:(cj + 1) * 128],
                            PT[:, 2 * j:2 * j + 2, qs],
                            start=(j == 0), stop=(j == NKB // 2 - 1),
                            perf_mode=DR)
                for j in range(NKB // 2):
                    nc.tensor.matmul(zcols[:, qt:qt + 1],
                                     PT[:, 2 * j:2 * j + 2, qs], ones2,
                                     start=(j == 0), stop=(j == NKB // 2 - 1),
                                     perf_mode=DR)
                nc.vector.reciprocal(out=rcp_all[:, qt:qt + 1],
                                     in_=zcols[:, qt:qt + 1])
                return pv

            aos = {}

            def fin_part(qt):
                """fp8 cast + proj + 1/Z + residual + store for qtile qt.
                Tail qtiles route the psum reads via the then-idle Act."""
                pv = aos.pop(qt)
                # late fins ride the idle Act, except the very last one:
                # the DVE path has one hop less on the final drain
                tail = 12 <= qt < NQT - 1
                atsb = work.tile([128, NCH, 128], FP8, tag="atsb", bufs=2,
                                 name=f"atsb{qt}")
                if tail:
                    nc.scalar.activation(
                        out=atsb.rearrange("p c q -> p (c q)"), in_=pv,
                        func=AF.Identity, scale=DS)
                else:
                    nc.vector.tensor_scalar_mul(
                        out=atsb.rearrange("p c q -> p (c q)"), in0=pv,
                        scalar1=DS)
                pj = ps.tile([128, C], F32, tag="pj", name=f"pj{qt}")
                for u in range(2):
                    nc.tensor.matmul(
                        pj, atsb[:, 2 * u:2 * u + 2, :],
                        w_sb["wp"][:, 2 * u:2 * u + 2, :],
                        start=(u == 0), stop=(u == 1), perf_mode=DR,
                    )
                o_sb = work.tile([128, C], F32, tag="osb", name=f"osb{qt}")
                if tail:  # Act scales by 1/Z, Pool adds the residual (sbuf)
                    tm = work.tile([128, C], BF16, tag="tm", bufs=2,
                                   name=f"tm{qt}")
                    nc.scalar.activation(out=tm, in_=pj, func=AF.Identity,
                                         scale=rcp_all[:, qt:qt + 1])
                    nc.gpsimd.tensor_add(o_sb, tm, xq_all[:, qt, :])
                else:
                    nc.vector.scalar_tensor_tensor(
                        out=o_sb, in0=pj, scalar=rcp_all[:, qt:qt + 1],
                        in1=xq_all[:, qt, :], op0=ALU.mult, op1=ALU.add,
                    )
                nc.sync.dma_start(out=out[qt * 128:(qt + 1) * 128, :], in_=o_sb)

            # Remaining normalize + kT gemms.  The b4=1..3 norms become ready
            # the moment rstd lands but are only needed from S^T kb8 on;
            # tile_wait_until keeps them out of the critical head window
            # (the scheduler orders per-engine by readiness, not priority).
            for b4 in range(1, 4):
                with tc.tile_wait_until(0.024 + 0.002 * b4):
                    for ci in range(NCH):
                        norm_block(ci, b4, nc.gpsimd)
                for sl in (2 * b4, 2 * b4 + 1):
                    for co in range(NCH):
                        kq_gemm("wk", kT, bk_sb, sl, co, nc.vector)
            for sl in range(2, 4):                # qT chunk 1 (epi DVE)
                for co in range(NCH):
                    kq_gemm("wq", qT, bq_sb, sl, co, nc.vector)
            # the S^T + exp critical stream, both chunks back to back
            for chunk in range(NCHUNK):
                for kb in range(NKB):
                    st_exp(chunk, kb)
            # V gemm band: consumed by pv_parts (first use ~62us); held past
            # the head so its epis can't interleave into the kq epi stream.
            with tc.high_priority(offset=-100000), tc.tile_wait_until(0.028):
                for kb in range(NKB):
                    v_gemm(kb, nc.vector)
            # attention epilogue band (+B2): PV/Z, rcp/ao, transp/proj/resid
            bfin_rep = bass.AP(tensor=bfin_bc.tensor, offset=bfin_bc.offset,
                               ap=[bfin_bc.ap[0], [0, 4], bfin_bc.ap[1]])
            with tc.high_priority(offset=-150000):
                for qt in range(NQT):             # xq_pre = x + bfin (Pool,
                    nc.gpsimd.tensor_add(         # sbuf-only, no broadcast AP)
                        xq_all[:, qt, :], xq_all[:, qt, :], bfin_bc)
                for qt in range(NQT):
                    # chunk0 pvs alternate pv/at (2-deep: the V-gemm "at"
                    # bank is free by then) so each pv doesn't serialize on
                    # the previous one's DVE drain; tail pvs use the freed
                    # S^T banks.
                    tag = ("pv", "at")[qt % 2] if qt < NQT // 2 else "sT"
                    aos[qt] = pv_part(qt, pv_tag=tag)
                    fin_part(qt)

    if spill:
        _spill_excess_waits(nc)
    return nc


def _spill_excess_waits(nc):
    """Walrus enforces tight per-instruction sync-wait slot limits (1 for
    most opcodes, 2 for EventSemaphore).  Tile's sem assignment occasionally
    emits more at multi-producer joins; spill the excess onto same-engine
    EventSemaphore ops inserted immediately before the offender."""
    n = 0
    for fn in nc.m.functions:
        for bb in fn.blocks:
            out_insts = []
            changed = False
            for inst in bb.instructions:
                si = inst.sync_info
                waits = list(si.on_wait) if si is not None and si.on_wait else []
                cap = 2 if inst.__class__.__name__ == "InstEventSemaphore" else 1
                if len(waits) > cap:
                    keep = waits[-cap:]
                    excess = waits[:-cap]
                    for j in range(0, len(excess), 2):
                        n += 1
                        es = mybir.InstEventSemaphore(
                            name=f"W-spill-{n}",
                            engine=inst.engine,
                            ins=[], outs=[],
                            sync_info=mybir.SyncInfo(
                                on_wait=excess[j:j + 2], on_update=[]
                            ),
                        )
                        out_insts.append(es)
                    si.on_wait = keep
                    changed = True
                out_insts.append(inst)
            if changed:
                bb.instructions = out_insts
    return n


def _prepare(x, gamma, beta, wq, bq, wk, bk, wv, bv, wp, bp):
    f32 = np.float32
    f8 = ml_dtypes.float8_e4m3
    x = np.asarray(x, f32)
    gamma = np.asarray(gamma, f32)
    beta = np.asarray(beta, f32)
    scale = f32(1.0 / np.sqrt(C))

    def permci(w):  # c_in chunk rows reordered to (1,2,3,0); see POS in build
        return np.ascontiguousarray(
            w.reshape(4, 128, C)[[1, 2, 3, 0]].reshape(C, C))

    wq_f = permci((gamma[:, None] * np.asarray(wq, f32)) * scale)
    bq_f = (beta @ np.asarray(wq, f32) + np.asarray(bq, f32)) * scale
    wk_f = permci(gamma[:, None] * np.asarray(wk, f32))
    bk_f = beta @ np.asarray(wk, f32) + np.asarray(bk, f32)
    wv_f = permci(gamma[:, None] * np.asarray(wv, f32))
    bv_f = beta @ np.asarray(wv, f32) + np.asarray(bv, f32)
    wp_f = np.asarray(wp, f32)
    bfin = bv_f @ wp_f + np.asarray(bp, f32)
    gm = (np.kron(np.eye(8, dtype=f32), np.ones((16, 16), f32)) / 16.0)
    common = dict(
        wq=(wq_f * W8SCALE).astype(f8), wk=(wk_f * W8SCALE).astype(f8),
        wv=(wv_f * W8SCALE).astype(f8), wp=(wp_f * W8SCALE).astype(f8),
        bq=bq_f.astype(f32), bk=bk_f.astype(f32),
        bfin=bfin.astype(f32), gmask=gm,
    )
    in_maps = []
    for b in range(4):
        xb = np.ascontiguousarray(x[b].reshape(PIX, C))
        for hh in range(2):
            xp = xb if hh == 0 else np.concatenate([xb[QPIX:], xb[:QPIX]])
            xp = np.ascontiguousarray(xp)
            xtp = np.ascontiguousarray(xp.T).astype(f8)
            in_maps.append(dict(common, x=np.ascontiguousarray(xp[:QPIX]),
                                xt=xtp))
    return in_maps


def kernel(x, gamma, beta, wq, bq, wk, bk, wv, bv, wp, bp, _trace=False):
    from concourse.bass_utils import run_bass_kernel_spmd

    if "nc" not in _CACHED:
        _CACHED["nc"] = build_program()
    nc = _CACHED["nc"]
    in_maps = _prepare(x, gamma, beta, wq, bq, wk, bk, wv, bv, wp, bp)
    res = run_bass_kernel_spmd(nc, in_maps, list(range(8)), trace=_trace)
    _CACHED["last_results"] = res
    out = np.empty((4, PIX, C), np.float32)
    for core in range(8):
        b, hh = divmod(core, 2)
        out[b, hh * QPIX:(hh + 1) * QPIX] = res.results[core]["out"]
    return out.reshape(4, 64, 64, C)
